# revision 8
# baseline (speedup 1.0000x reference)
"""NetVLAD consensus kernel for Trainium2 (8 NeuronCores, SPMD data-parallel).

Full-input contract: kernel(x, W, b, centroids) -> [32, 32768] fp32.

Sharding: data-parallel over batch N=32 -> 4 items per core; W/b/centroids
replicated. Per item:
  logitsT[k,t] = sum_c W[k,c] x[t,c]   (PE, contract C in 4 chunks of 128)
  e = exp(logitsT + b)                 (ACT, per-partition bias)
  eT tiles [t,k] via PE transpose; softmax normalize on DVE
  vlad[k,c] = sum_t a[t,k] x[t,c]      (PE, accumulate 8 t-tiles in PSUM)
  asum[k]   = sum_t a[t,k]             (PE, N=1 matmul vs ones)
  vlad -= asum*centroids; intra-L2-norm; global-L2-norm  (DVE/ACT + tiny PE)

x is needed C-major for the logits contraction and T-major for the VLAD
contraction, so the host passes both layouts (each read once on device).
"""

import numpy as np
import ml_dtypes

import concourse.bass as bass
import concourse.tile as tile
from concourse import bacc, mybir
from concourse.bass_utils import run_bass_kernel_spmd

N, T, C, K = 32, 1024, 512, 64
NCORES = 8
NB = N // NCORES          # batch items per core
TT = 128                  # t-tile (partition dim for transposed ops)
TG = 512                  # t-group (matmul moving free dim)
NG = T // TG              # t-groups per item
NTT = T // TT             # t-tiles per item
NCC = C // 128            # c-chunks (contraction tiles) per item
EPS = 1e-12

f32 = mybir.dt.float32
f32r = mybir.dt.float32r
bf16 = mybir.dt.bfloat16

# dtype knobs: "f32r" or "bf16" for the two big matmuls
LOGITS_DT = "bf16"
VLAD_DT = "f32r"


def _mm_dt(which):
    """Tile/DRAM dtype for a matmul operand path.

    float32r is fp32 with the low 12 mantissa bits zeroed (TF32-like, 11
    explicit mantissa bits); it runs at full PE rate for moving dim >= 256.
    The walrus verifier requires the whole producer chain to be f32r-typed,
    so tensors are declared f32r and the host pre-rounds onto the f32r grid.
    """
    return bf16 if which == "bf16" else f32r


def _np_prep(arr, which):
    if which == "bf16":
        return np.asarray(arr, np.float32).astype(ml_dtypes.bfloat16)
    return round_fp32r(np.asarray(arr, np.float32))


def round_fp32r(a):
    """Round fp32 array to nearest-even on the fp32r grid (low 12 bits zero)."""
    u = np.ascontiguousarray(a, np.float32).view(np.uint32)
    keep = u & np.uint32(0xFFFFF000)
    rem = u & np.uint32(0xFFF)
    lsb = (u >> np.uint32(12)) & np.uint32(1)
    roundup = (rem > 0x800) | ((rem == 0x800) & (lsb == 1))
    out = keep + (roundup.astype(np.uint32) << np.uint32(12))
    return out.view(np.float32)


def build_program(reps=1):
    """Build the SPMD Bass program (one core's view; same program all cores)."""
    from contextlib import ExitStack

    nc = bacc.Bacc("TRN2", target_bir_lowering=False, debug=False,
                   num_devices=NCORES)

    ldt = _mm_dt(LOGITS_DT)
    vdt = _mm_dt(VLAD_DT)

    x_d = nc.dram_tensor("x", [NB, T, C], vdt, kind="ExternalInput")
    xt_d = nc.dram_tensor("xT", [NB, C, T], ldt, kind="ExternalInput")
    wt_d = nc.dram_tensor("WT", [C, K], ldt, kind="ExternalInput")
    b_d = nc.dram_tensor("b", [K, 1], f32, kind="ExternalInput")
    cent_d = nc.dram_tensor("cent", [K, C], f32, kind="ExternalInput")
    id_d = nc.dram_tensor("ident", [K, K], f32, kind="ExternalInput")
    out_d = nc.dram_tensor("out", [NB, K * C], f32, kind="ExternalOutput")

    with tile.TileContext(nc) as tc:
        with ExitStack() as ctx:
            _body(ctx, tc, nc, x_d, xt_d, wt_d, b_d, cent_d, id_d, out_d, reps)
    nc.compile()
    return nc


def _body(ctx, tc, nc, x_d, xt_d, wt_d, b_d, cent_d, id_d, out_d, reps):
    ldt = _mm_dt(LOGITS_DT)
    vdt = _mm_dt(VLAD_DT)
    X = mybir.AxisListType.X
    Exp = mybir.ActivationFunctionType.Exp
    Sqrt = mybir.ActivationFunctionType.Sqrt
    Square = mybir.ActivationFunctionType.Square
    mult = mybir.AluOpType.mult
    add = mybir.AluOpType.add

    consts = ctx.enter_context(tc.tile_pool(name="consts", bufs=1))
    io = ctx.enter_context(tc.tile_pool(name="io", bufs=2))
    work = ctx.enter_context(tc.tile_pool(name="work", bufs=3))
    ps_vl = ctx.enter_context(tc.tile_pool(name="ps_vl", bufs=2, space="PSUM"))
    ps_lg = ctx.enter_context(tc.tile_pool(name="ps_lg", bufs=2, space="PSUM"))
    ps_eT = ctx.enter_context(tc.tile_pool(name="ps_eT", bufs=2, space="PSUM"))
    ps_sm = ctx.enter_context(tc.tile_pool(name="ps_sm", bufs=2, space="PSUM"))

    # --- constants (loaded once) ---
    wt_sb = consts.tile([128, NCC, K], ldt)              # W^T c-chunks
    nc.sync.dma_start(wt_sb[:], wt_d.ap().rearrange("(cc p) k -> p cc k", p=128))
    b_sb = consts.tile([K, 1], f32)
    nc.sync.dma_start(b_sb[:], b_d.ap())
    cent_sb = consts.tile([K, C], f32)
    nc.sync.dma_start(cent_sb[:], cent_d.ap())
    id_sb = consts.tile([K, K], f32)
    nc.sync.dma_start(id_sb[:], id_d.ap())
    # ones column for the asum matmul. That matmul has N=1 (odd), which the
    # f32r ISA pattern rules reject, so it runs in plain fp32/bf16 instead
    # (f32r bits are valid fp32 bits, so a bitcast view is exact).
    ones_col = consts.tile([128, 1], bf16 if VLAD_DT == "bf16" else f32)
    nc.vector.memset(ones_col[:], 1.0)
    ones_f32 = consts.tile([K, 1], f32)
    nc.vector.memset(ones_f32[:], 1.0)
    ones_row = consts.tile([1, K], f32)
    nc.vector.memset(ones_row[:], 1.0)

    for rep in range(reps):
        for n in range(NB):
            # --- load this item's two x layouts ---
            xb = io.tile([128, NTT, C], vdt, tag="xb")
            nc.sync.dma_start(
                xb[:], x_d.ap()[n].rearrange("(g p) c -> p g c", p=128))
            xtb = io.tile([128, NCC, T], ldt, tag="xtb")
            nc.sync.dma_start(
                xtb[:], xt_d.ap()[n].rearrange("(cc p) t -> p cc t", p=128))

            vl_ps = ps_vl.tile([K, C], f32, tag="vl")
            as_ps = ps_sm.tile([K, 1], f32, tag="sm")

            for g in range(NG):
                # logitsT [k, t-group] accumulated over c-chunks
                lg_ps = ps_lg.tile([K, TG], f32, tag="lg")
                for cc in range(NCC):
                    nc.tensor.matmul(
                        lg_ps[:],
                        wt_sb[:, cc, :],
                        xtb[:, cc, bass.ts(g, TG)],
                        start=(cc == 0), stop=(cc == NCC - 1))
                # e = exp(logitsT + b)
                e_sb = work.tile([K, TG], f32, tag="e")
                nc.scalar.activation(e_sb[:], lg_ps[:], Exp, bias=b_sb[:])

                # transpose to [t, k] tiles (4 per group, one PSUM bank)
                eT_ps = ps_eT.tile([TT, TG // TT, K], f32, tag="eT")
                for j in range(TG // TT):
                    nc.tensor.transpose(
                        eT_ps[:, j, :], e_sb[:, bass.ts(j, TT)], id_sb[:])

                # softmax normalize: a = e / colsum(e)
                s_col = work.tile([TT, TG // TT, 1], f32, tag="s")
                nc.vector.reduce_sum(s_col[:], eT_ps[:], axis=X)
                rs_col = work.tile([TT, TG // TT, 1], f32, tag="rs")
                nc.vector.reciprocal(rs_col[:], s_col[:])
                a_sb = work.tile([TT, TG // TT, K], vdt, tag="a")
                nc.vector.tensor_tensor(
                    out=a_sb[:], in0=eT_ps[:],
                    in1=rs_col[:].broadcast_to([TT, TG // TT, K]), op=mult)

                # VLAD accumulation over t-tiles
                for j in range(TG // TT):
                    ti = g * (TG // TT) + j
                    nc.tensor.matmul(
                        vl_ps[:],
                        a_sb[:, j, :],
                        xb[:, ti, :],
                        start=(ti == 0), stop=(ti == NTT - 1))
                    a_for_sum = (a_sb[:, j, :] if VLAD_DT == "bf16"
                                 else a_sb[:, j, :].bitcast(f32))
                    nc.tensor.matmul(
                        as_ps[:],
                        a_for_sum,
                        ones_col[:],
                        start=(ti == 0), stop=(ti == NTT - 1))

            # --- finalize item: centroid subtract + intra/global L2 norm ---
            nas = work.tile([K, 1], f32, tag="nas")
            nc.vector.tensor_scalar_mul(nas[:], as_ps[:], -1.0)
            vlad_sb = work.tile([K, C], f32, tag="vlad")
            nc.vector.scalar_tensor_tensor(
                out=vlad_sb[:], in0=cent_sb[:], scalar=nas[:], in1=vl_ps[:],
                op0=mult, op1=add)

            sq = work.tile([K, C], f32, tag="sq")
            ss = work.tile([K, 1], f32, tag="ss")
            nc.scalar.activation(sq[:], vlad_sb[:], Square, accum_out=ss[:])
            norm = work.tile([K, 1], f32, tag="norm")
            nc.scalar.activation(norm[:], ss[:], Sqrt)
            normc = work.tile([K, 1], f32, tag="normc")
            nc.vector.tensor_scalar_max(normc[:], norm[:], EPS)
            rnorm = work.tile([K, 1], f32, tag="rnorm")
            nc.vector.reciprocal(rnorm[:], normc[:])

            # global norm^2 = sum_k ss_k * rnorm_k^2 (rows are unit after intra-norm)
            q = work.tile([K, 1], f32, tag="q")
            nc.vector.tensor_scalar(
                out=q[:], in0=ss[:], scalar1=rnorm[:], scalar2=rnorm[:],
                op0=mult, op1=mult)
            g_ps = ps_sm.tile([1, 1], f32, tag="sm")
            nc.tensor.matmul(g_ps[:], q[:], ones_f32[:], start=True, stop=True)
            rgs = work.tile([1, 1], f32, tag="rgs")
            nc.vector.reciprocal(rgs[:], g_ps[:])
            rg = work.tile([1, 1], f32, tag="rg")
            nc.scalar.activation(rg[:], rgs[:], Sqrt)
            # broadcast 1/gnorm across partitions, fold with rnorm
            bc_ps = ps_sm.tile([K, 1], f32, tag="sm")
            nc.tensor.matmul(bc_ps[:], ones_row[:], rg[:], start=True, stop=True)
            scale = work.tile([K, 1], f32, tag="scale")
            nc.vector.tensor_tensor(out=scale[:], in0=rnorm[:], in1=bc_ps[:],
                                    op=mult)
            outt = work.tile([K, C], f32, tag="outt")
            nc.vector.tensor_scalar_mul(outt[:], vlad_sb[:], scale[:])
            nc.sync.dma_start(
                out_d.ap()[n].rearrange("(k c) -> k c", k=K), outt[:])


_NC_CACHE = {}


def _get_program(reps=1):
    if reps not in _NC_CACHE:
        _NC_CACHE[reps] = build_program(reps)
    return _NC_CACHE[reps]


def make_in_maps(x, W, b, centroids):
    x = np.asarray(x, dtype=np.float32)
    xT = _np_prep(np.ascontiguousarray(x.transpose(0, 2, 1)), LOGITS_DT)
    xv = _np_prep(x, VLAD_DT)
    WT = _np_prep(np.ascontiguousarray(np.asarray(W, np.float32).T), LOGITS_DT)
    bcol = np.asarray(b, np.float32).reshape(K, 1)
    cent = np.asarray(centroids, np.float32)
    ident = np.eye(K, dtype=np.float32)
    return [
        dict(x=xv[i * NB:(i + 1) * NB], xT=xT[i * NB:(i + 1) * NB],
             WT=WT, b=bcol, cent=cent, ident=ident)
        for i in range(NCORES)
    ]


def kernel(x, W, b, centroids):
    nc = _get_program()
    in_maps = make_in_maps(x, W, b, centroids)
    res = run_bass_kernel_spmd(nc, in_maps, list(range(NCORES)))
    return np.concatenate([res.results[i]["out"] for i in range(NCORES)],
                          axis=0).reshape(N, K * C)


# revision 10
# speedup vs baseline: 1.3484x; 1.3484x over previous
"""NetVLAD consensus kernel for Trainium2 (8 NeuronCores, SPMD data-parallel).

Full-input contract: kernel(x, W, b, centroids) -> [32, 32768] fp32.

Sharding: data-parallel over batch N=32 -> 4 items per core; W/b/centroids
replicated. Per item:
  logitsT[k,t] = sum_c W[k,c] x[t,c]   (PE, contract C in 4 chunks of 128)
  e = exp(logitsT + b)                 (ACT, per-partition bias)
  eT tiles [t,k] via PE transpose; softmax normalize on DVE
  vlad[k,c] = sum_t a[t,k] x[t,c]      (PE, accumulate 8 t-tiles in PSUM)
  vlad -= asum*centroids; intra-L2-norm; global scale     (DVE + tail ACT)

Key layout/perf choices:
- x is needed C-major for the logits contraction and T-major for the VLAD
  contraction, so the host passes both layouts in bf16 (each read once).
- asum[k] = sum_t a[t,k] is folded into the VLAD matmul: the host appends a
  ones column to x, and the VLAD matmul is split into N=257 / N=256 halves
  (PSUM bank limit). No separate N=1 matmuls.
- After intra-normalization every row has unit L2 norm, so the global norm
  equals sqrt(K) = 8 up to fp32 rounding (~1e-7); the final scale uses the
  constant 1/8.
- All sqrt work is deferred to one tail phase so the scalar engine loads the
  Exp and Sqrt activation tables once each (table load is ~1.3us).
"""

import numpy as np
import ml_dtypes
from contextlib import ExitStack

import concourse.bass as bass
import concourse.tile as tile
from concourse import bacc, mybir
from concourse.bass_utils import run_bass_kernel_spmd

N, T, C, K = 32, 1024, 512, 64
NCORES = 8
NB = N // NCORES          # batch items per core
TT = 128                  # t-tile (partition dim for transposed ops)
TG = 512                  # t-group (logits matmul moving free dim)
NG = T // TG              # t-groups per item
NTT = T // TT             # t-tiles per item
NCC = C // 128            # c-chunks (contraction tiles)
CPAD = C + 2              # x augmented with a ones column (+ zero pad)
CA = C // 2 + 1           # first VLAD half: c 0..255 + asum column
CB = C // 2               # second VLAD half: c 256..511
EPS = 1e-12

f32 = mybir.dt.float32
bf16 = mybir.dt.bfloat16


def round_fp32r(a):
    """Round fp32 array to nearest-even on the fp32r grid (low 12 bits zero)."""
    u = np.ascontiguousarray(a, np.float32).view(np.uint32)
    keep = u & np.uint32(0xFFFFF000)
    rem = u & np.uint32(0xFFF)
    lsb = (u >> np.uint32(12)) & np.uint32(1)
    roundup = (rem > 0x800) | ((rem == 0x800) & (lsb == 1))
    out = keep + (roundup.astype(np.uint32) << np.uint32(12))
    return out.view(np.float32)


def build_program(reps=1):
    """Build the SPMD Bass program (one core's view; same program all cores)."""
    nc = bacc.Bacc("TRN2", target_bir_lowering=False, debug=False,
                   num_devices=NCORES)

    x_d = nc.dram_tensor("x", [NB, T, CPAD], bf16, kind="ExternalInput")
    xt_d = nc.dram_tensor("xT", [NB, C, T], bf16, kind="ExternalInput")
    wt_d = nc.dram_tensor("WT", [C, K], bf16, kind="ExternalInput")
    b_d = nc.dram_tensor("b", [K, 1], f32, kind="ExternalInput")
    cent_d = nc.dram_tensor("cent", [K, C], f32, kind="ExternalInput")
    id_d = nc.dram_tensor("ident", [K, K], f32, kind="ExternalInput")
    out_d = nc.dram_tensor("out", [NB, K * C], f32, kind="ExternalOutput")

    with tile.TileContext(nc) as tc:
        with ExitStack() as ctx:
            _body(ctx, tc, nc, x_d, xt_d, wt_d, b_d, cent_d, id_d, out_d, reps)
    nc.compile()
    return nc


def _body(ctx, tc, nc, x_d, xt_d, wt_d, b_d, cent_d, id_d, out_d, reps):
    X = mybir.AxisListType.X
    Exp = mybir.ActivationFunctionType.Exp
    Sqrt = mybir.ActivationFunctionType.Sqrt
    mult = mybir.AluOpType.mult
    add = mybir.AluOpType.add

    consts = ctx.enter_context(tc.tile_pool(name="consts", bufs=1))
    io = ctx.enter_context(tc.tile_pool(name="io", bufs=2))
    work = ctx.enter_context(tc.tile_pool(name="work", bufs=3))
    keep = ctx.enter_context(tc.tile_pool(name="keep", bufs=NB))
    ps_vl = ctx.enter_context(tc.tile_pool(name="ps_vl", bufs=2, space="PSUM"))
    ps_lg = ctx.enter_context(tc.tile_pool(name="ps_lg", bufs=2, space="PSUM"))
    ps_eT = ctx.enter_context(tc.tile_pool(name="ps_eT", bufs=2, space="PSUM"))

    # --- constants (loaded once) ---
    wt_sb = consts.tile([128, NCC, K], bf16)             # W^T c-chunks
    nc.sync.dma_start(wt_sb[:], wt_d.ap().rearrange("(cc p) k -> p cc k", p=128))
    b_sb = consts.tile([K, 1], f32)
    nc.sync.dma_start(b_sb[:], b_d.ap())
    cent_sb = consts.tile([K, C], f32)
    nc.sync.dma_start(cent_sb[:], cent_d.ap())
    id_sb = consts.tile([K, K], f32)
    nc.sync.dma_start(id_sb[:], id_d.ap())

    for rep in range(reps):
        finals = []
        ss_all = keep.tile([K, NB], f32, tag="ss_all", bufs=1)
        for n in range(NB):
            # --- load this item's two x layouts ---
            xb = io.tile([128, NTT, CPAD], bf16, tag="xb")
            nc.sync.dma_start(
                xb[:], x_d.ap()[n].rearrange("(g p) c -> p g c", p=128))
            xtb = io.tile([128, NCC, T], bf16, tag="xtb")
            nc.sync.dma_start(
                xtb[:], xt_d.ap()[n].rearrange("(cc p) t -> p cc t", p=128))

            # vl_a holds c 0..255 plus the asum column (col 256); vl_b the rest
            vl_a = ps_vl.tile([K, CA], f32, tag="vl_a")
            vl_b = ps_vl.tile([K, CB], f32, tag="vl_b")

            for g in range(NG):
                # logitsT [k, t-group] accumulated over c-chunks
                lg_ps = ps_lg.tile([K, TG], f32, tag="lg")
                for cc in range(NCC):
                    nc.tensor.matmul(
                        lg_ps[:],
                        wt_sb[:, cc, :],
                        xtb[:, cc, bass.ts(g, TG)],
                        start=(cc == 0), stop=(cc == NCC - 1))
                # e = exp(logitsT + b)
                e_sb = work.tile([K, TG], f32, tag="e")
                nc.scalar.activation(e_sb[:], lg_ps[:], Exp, bias=b_sb[:])

                # transpose to [t, k] tiles (4 per group, one PSUM bank)
                eT_ps = ps_eT.tile([TT, TG // TT, K], f32, tag="eT")
                for j in range(TG // TT):
                    nc.tensor.transpose(
                        eT_ps[:, j, :], e_sb[:, bass.ts(j, TT)], id_sb[:])

                # softmax normalize: a = e / colsum(e)
                s_col = work.tile([TT, TG // TT, 1], f32, tag="s")
                nc.vector.reduce_sum(s_col[:], eT_ps[:], axis=X)
                rs_col = work.tile([TT, TG // TT, 1], f32, tag="rs")
                nc.vector.reciprocal(rs_col[:], s_col[:])
                a_sb = work.tile([TT, TG // TT, K], bf16, tag="a")
                nc.vector.tensor_tensor(
                    out=a_sb[:], in0=eT_ps[:],
                    in1=rs_col[:].broadcast_to([TT, TG // TT, K]), op=mult)

                # VLAD accumulation over t-tiles (split N=257/256; the ones
                # column of x makes vl_a[:, 256] the asum accumulator)
                for j in range(TG // TT):
                    ti = g * (TG // TT) + j
                    nc.tensor.matmul(
                        vl_a[:], a_sb[:, j, :], xb[:, ti, 0:CA],
                        start=(ti == 0), stop=(ti == NTT - 1))
                    nc.tensor.matmul(
                        vl_b[:], a_sb[:, j, :], xb[:, ti, CA:CA + CB],
                        start=(ti == 0), stop=(ti == NTT - 1))

            # --- per-item epilogue: centroid subtract + sum of squares ---
            nas = work.tile([K, 1], f32, tag="nas")
            nc.vector.tensor_scalar_mul(nas[:], vl_a[:, C // 2:C // 2 + 1], -1.0)
            vlad_sb = keep.tile([K, C], f32, tag="vlad")
            nc.vector.scalar_tensor_tensor(
                out=vlad_sb[:, 0:C // 2], in0=cent_sb[:, 0:C // 2],
                scalar=nas[:], in1=vl_a[:, 0:C // 2], op0=mult, op1=add)
            nc.vector.scalar_tensor_tensor(
                out=vlad_sb[:, C // 2:C], in0=cent_sb[:, C // 2:C],
                scalar=nas[:], in1=vl_b[:], op0=mult, op1=add)
            sq = work.tile([K, C], f32, tag="sq")
            nc.vector.scalar_tensor_tensor(
                out=sq[:], in0=vlad_sb[:], scalar=1.0, in1=vlad_sb[:],
                op0=mult, op1=mult, accum_out=ss_all[:, n:n + 1])
            finals.append(vlad_sb)

        # --- tail: all sqrt work batched (one Sqrt table load), then scale.
        # After intra-normalization each row is unit, so the global norm is
        # sqrt(K)=8 up to fp32 rounding; the final scale is rnorm/8.
        norm = work.tile([K, NB], f32, tag="norm")
        nc.scalar.activation(norm[:], ss_all[:], Sqrt)
        normc = work.tile([K, NB], f32, tag="normc")
        nc.vector.tensor_scalar_max(normc[:], norm[:], EPS)
        rnorm = work.tile([K, NB], f32, tag="rnorm")
        nc.vector.reciprocal(rnorm[:], normc[:])
        for n in range(NB):
            outt = work.tile([K, C], f32, tag="outt")
            nc.vector.tensor_scalar(
                out=outt[:], in0=finals[n][:], scalar1=rnorm[:, n:n + 1],
                scalar2=1.0 / 8.0, op0=mult, op1=mult)
            nc.sync.dma_start(
                out_d.ap()[n].rearrange("(k c) -> k c", k=K), outt[:])


_NC_CACHE = {}


def _get_program(reps=1):
    if reps not in _NC_CACHE:
        _NC_CACHE[reps] = build_program(reps)
    return _NC_CACHE[reps]


def make_in_maps(x, W, b, centroids):
    x = np.asarray(x, dtype=np.float32)
    xaug = np.zeros((N, T, CPAD), dtype=ml_dtypes.bfloat16)
    xaug[:, :, :C] = x.astype(ml_dtypes.bfloat16)
    xaug[:, :, C] = 1.0
    # reorder so device slice [0:257] is c 0..255 + ones, [257:513] is c 256..511
    perm = list(range(C // 2)) + [C] + list(range(C // 2, C)) + [C + 1]
    xaug = np.ascontiguousarray(xaug[:, :, perm])
    xT = np.ascontiguousarray(x.transpose(0, 2, 1)).astype(ml_dtypes.bfloat16)
    WT = np.ascontiguousarray(np.asarray(W, np.float32).T).astype(ml_dtypes.bfloat16)
    bcol = np.asarray(b, np.float32).reshape(K, 1)
    cent = np.asarray(centroids, np.float32)
    ident = np.eye(K, dtype=np.float32)
    return [
        dict(x=xaug[i * NB:(i + 1) * NB], xT=xT[i * NB:(i + 1) * NB],
             WT=WT, b=bcol, cent=cent, ident=ident)
        for i in range(NCORES)
    ]


def kernel(x, W, b, centroids):
    nc = _get_program()
    in_maps = make_in_maps(x, W, b, centroids)
    res = run_bass_kernel_spmd(nc, in_maps, list(range(NCORES)))
    return np.concatenate([res.results[i]["out"] for i in range(NCORES)],
                          axis=0).reshape(N, K * C)


# revision 11
# speedup vs baseline: 1.5614x; 1.1580x over previous
"""NetVLAD consensus kernel for Trainium2 (8 NeuronCores, SPMD data-parallel).

Full-input contract: kernel(x, W, b, centroids) -> [32, 32768] fp32.

Sharding: data-parallel over batch N=32 -> 4 items per core; W/b/centroids
replicated. Per item:
  logitsT[k,t] = sum_c W[k,c] x[t,c]   (PE, contract C in 4 chunks of 128)
  e = exp(logitsT + b)                 (ACT, per-partition bias)
  eT tiles [t,k] via PE transpose; softmax normalize on DVE
  vlad[k,c] = sum_t a[t,k] x[t,c]      (PE, accumulate 8 t-tiles in PSUM)
  vlad -= asum*centroids; intra-L2-norm; global scale     (DVE + tail ACT)

Key layout/perf choices:
- x is needed C-major for the logits contraction and T-major for the VLAD
  contraction, so the host passes both layouts in bf16 (each read once).
- asum[k] = sum_t a[t,k] is folded into the VLAD matmul: the host appends a
  ones column to x, and the VLAD matmul is split into N=257 / N=256 halves
  (PSUM bank limit). No separate N=1 matmuls.
- After intra-normalization every row has unit L2 norm, so the global norm
  equals sqrt(K) = 8 up to fp32 rounding (~1e-7); the final scale uses the
  constant 1/8.
- All sqrt work is deferred to one tail phase so the scalar engine loads the
  Exp and Sqrt activation tables once each (table load is ~1.3us).
"""

import numpy as np
import ml_dtypes
from contextlib import ExitStack

import concourse.bass as bass
import concourse.tile as tile
from concourse import bacc, mybir
from concourse.bass_utils import run_bass_kernel_spmd

N, T, C, K = 32, 1024, 512, 64
NCORES = 8
NB = N // NCORES          # batch items per core
TT = 128                  # t-tile (partition dim for transposed ops)
TG = 512                  # t-group (logits matmul moving free dim)
NG = T // TG              # t-groups per item
NTT = T // TT             # t-tiles per item
NCC = C // 128            # c-chunks (contraction tiles)
CPAD = C + 2              # x augmented with a ones column (+ zero pad)
CA = C // 2 + 1           # first VLAD half: c 0..255 + asum column
CB = C // 2               # second VLAD half: c 256..511
EPS = 1e-12

f32 = mybir.dt.float32
bf16 = mybir.dt.bfloat16


def round_fp32r(a):
    """Round fp32 array to nearest-even on the fp32r grid (low 12 bits zero)."""
    u = np.ascontiguousarray(a, np.float32).view(np.uint32)
    keep = u & np.uint32(0xFFFFF000)
    rem = u & np.uint32(0xFFF)
    lsb = (u >> np.uint32(12)) & np.uint32(1)
    roundup = (rem > 0x800) | ((rem == 0x800) & (lsb == 1))
    out = keep + (roundup.astype(np.uint32) << np.uint32(12))
    return out.view(np.float32)


def build_program(reps=1):
    """Build the SPMD Bass program (one core's view; same program all cores)."""
    nc = bacc.Bacc("TRN2", target_bir_lowering=False, debug=False,
                   num_devices=NCORES)

    x_d = nc.dram_tensor("x", [NB, T, CPAD], bf16, kind="ExternalInput")
    xt_d = nc.dram_tensor("xT", [NB, C, T], bf16, kind="ExternalInput")
    wt_d = nc.dram_tensor("WT", [C, K], bf16, kind="ExternalInput")
    b_d = nc.dram_tensor("b", [K, 1], f32, kind="ExternalInput")
    cent_d = nc.dram_tensor("cent", [K, C], f32, kind="ExternalInput")
    id_d = nc.dram_tensor("ident", [K, K], f32, kind="ExternalInput")
    out_d = nc.dram_tensor("out", [NB, K * C], f32, kind="ExternalOutput")

    with tile.TileContext(nc) as tc:
        with ExitStack() as ctx:
            _body(ctx, tc, nc, x_d, xt_d, wt_d, b_d, cent_d, id_d, out_d, reps)
    nc.compile()
    return nc


def _body(ctx, tc, nc, x_d, xt_d, wt_d, b_d, cent_d, id_d, out_d, reps):
    X = mybir.AxisListType.X
    Exp = mybir.ActivationFunctionType.Exp
    Sqrt = mybir.ActivationFunctionType.Sqrt
    mult = mybir.AluOpType.mult
    add = mybir.AluOpType.add

    consts = ctx.enter_context(tc.tile_pool(name="consts", bufs=1))
    io = ctx.enter_context(tc.tile_pool(name="io", bufs=2))
    work = ctx.enter_context(tc.tile_pool(name="work", bufs=3))
    keep = ctx.enter_context(tc.tile_pool(name="keep", bufs=NB))
    ps_vl = ctx.enter_context(tc.tile_pool(name="ps_vl", bufs=2, space="PSUM"))
    ps_lg = ctx.enter_context(tc.tile_pool(name="ps_lg", bufs=2, space="PSUM"))
    ps_eT = ctx.enter_context(tc.tile_pool(name="ps_eT", bufs=2, space="PSUM"))

    # --- constants (loaded once) ---
    wt_sb = consts.tile([128, NCC, K], bf16)             # W^T c-chunks
    nc.sync.dma_start(wt_sb[:], wt_d.ap().rearrange("(cc p) k -> p cc k", p=128))
    b_sb = consts.tile([K, 1], f32)
    nc.sync.dma_start(b_sb[:], b_d.ap())
    cent_sb = consts.tile([K, C], f32)
    nc.sync.dma_start(cent_sb[:], cent_d.ap())
    id_sb = consts.tile([K, K], f32)
    nc.sync.dma_start(id_sb[:], id_d.ap())

    for rep in range(reps):
        finals = []
        ss_all = keep.tile([K, NB], f32, tag="ss_all", bufs=1)
        out_all = keep.tile([K, NB, C], f32, tag="out_all", bufs=1)
        for n in range(NB):
            # --- load this item's two x layouts (separate DGE paths so the
            # descriptor-generation cost parallelizes) ---
            xb = io.tile([128, NTT, CPAD], bf16, tag="xb")
            nc.sync.dma_start(
                xb[:], x_d.ap()[n].rearrange("(g p) c -> p g c", p=128))
            xtb = io.tile([128, NCC, T], bf16, tag="xtb")
            nc.scalar.dma_start(
                xtb[:], xt_d.ap()[n].rearrange("(cc p) t -> p cc t", p=128))

            # vl_a holds c 0..255 plus the asum column (col 256); vl_b the rest
            vl_a = ps_vl.tile([K, CA], f32, tag="vl_a")
            vl_b = ps_vl.tile([K, CB], f32, tag="vl_b")
            # all 8 eT t-tiles of this item fill exactly one PSUM bank
            eT_ps = ps_eT.tile([TT, NTT, K], f32, tag="eT")

            for g in range(NG):
                # logitsT [k, t-group] accumulated over c-chunks
                lg_ps = ps_lg.tile([K, TG], f32, tag="lg")
                for cc in range(NCC):
                    nc.tensor.matmul(
                        lg_ps[:],
                        wt_sb[:, cc, :],
                        xtb[:, cc, bass.ts(g, TG)],
                        start=(cc == 0), stop=(cc == NCC - 1))
                # e = exp(logitsT + b)
                e_sb = work.tile([K, TG], f32, tag="e")
                nc.scalar.activation(e_sb[:], lg_ps[:], Exp, bias=b_sb[:])

                # transpose to [t, k] tiles (4 per group)
                for j in range(TG // TT):
                    nc.tensor.transpose(
                        eT_ps[:, g * (TG // TT) + j, :],
                        e_sb[:, bass.ts(j, TT)], id_sb[:])

            # softmax normalize for the whole item: a = e / colsum(e)
            s_col = work.tile([TT, NTT, 1], f32, tag="s")
            nc.vector.reduce_sum(s_col[:], eT_ps[:], axis=X)
            rs_col = work.tile([TT, NTT, 1], f32, tag="rs")
            nc.vector.reciprocal(rs_col[:], s_col[:])
            a_sb = work.tile([TT, NTT, K], bf16, tag="a")
            nc.vector.tensor_tensor(
                out=a_sb[:], in0=eT_ps[:],
                in1=rs_col[:].broadcast_to([TT, NTT, K]), op=mult)

            # VLAD accumulation over t-tiles (split N=257/256; the ones
            # column of x makes vl_a[:, 256] the asum accumulator)
            for ti in range(NTT):
                nc.tensor.matmul(
                    vl_a[:], a_sb[:, ti, :], xb[:, ti, 0:CA],
                    start=(ti == 0), stop=(ti == NTT - 1))
                nc.tensor.matmul(
                    vl_b[:], a_sb[:, ti, :], xb[:, ti, CA:CA + CB],
                    start=(ti == 0), stop=(ti == NTT - 1))

            # --- per-item epilogue: centroid subtract + sum of squares ---
            nas = work.tile([K, 1], f32, tag="nas")
            nc.vector.tensor_scalar_mul(nas[:], vl_a[:, C // 2:C // 2 + 1], -1.0)
            vlad_sb = keep.tile([K, C], f32, tag="vlad")
            nc.vector.scalar_tensor_tensor(
                out=vlad_sb[:, 0:C // 2], in0=cent_sb[:, 0:C // 2],
                scalar=nas[:], in1=vl_a[:, 0:C // 2], op0=mult, op1=add)
            nc.vector.scalar_tensor_tensor(
                out=vlad_sb[:, C // 2:C], in0=cent_sb[:, C // 2:C],
                scalar=nas[:], in1=vl_b[:], op0=mult, op1=add)
            sq = work.tile([K, C], f32, tag="sq")
            nc.vector.scalar_tensor_tensor(
                out=sq[:], in0=vlad_sb[:], scalar=1.0, in1=vlad_sb[:],
                op0=mult, op1=mult, accum_out=ss_all[:, n:n + 1])
            finals.append(vlad_sb)

        # --- tail: all sqrt work batched (one Sqrt table load), then scale.
        # After intra-normalization each row is unit, so the global norm is
        # sqrt(K)=8 up to fp32 rounding; the final scale is rnorm/8.
        norm = work.tile([K, NB], f32, tag="norm")
        nc.scalar.activation(norm[:], ss_all[:], Sqrt)
        normc = work.tile([K, NB], f32, tag="normc")
        nc.vector.tensor_scalar_max(normc[:], norm[:], EPS)
        rnorm = work.tile([K, NB], f32, tag="rnorm")
        nc.vector.reciprocal(rnorm[:], normc[:])
        for n in range(NB):
            nc.vector.tensor_scalar(
                out=out_all[:, n, :], in0=finals[n][:],
                scalar1=rnorm[:, n:n + 1],
                scalar2=1.0 / 8.0, op0=mult, op1=mult)
        nc.gpsimd.dma_start(
            out_d.ap().rearrange("n (k c) -> k n c", k=K), out_all[:])


_NC_CACHE = {}


def _get_program(reps=1):
    if reps not in _NC_CACHE:
        _NC_CACHE[reps] = build_program(reps)
    return _NC_CACHE[reps]


def make_in_maps(x, W, b, centroids):
    x = np.asarray(x, dtype=np.float32)
    xaug = np.zeros((N, T, CPAD), dtype=ml_dtypes.bfloat16)
    xaug[:, :, :C] = x.astype(ml_dtypes.bfloat16)
    xaug[:, :, C] = 1.0
    # reorder so device slice [0:257] is c 0..255 + ones, [257:513] is c 256..511
    perm = list(range(C // 2)) + [C] + list(range(C // 2, C)) + [C + 1]
    xaug = np.ascontiguousarray(xaug[:, :, perm])
    xT = np.ascontiguousarray(x.transpose(0, 2, 1)).astype(ml_dtypes.bfloat16)
    WT = np.ascontiguousarray(np.asarray(W, np.float32).T).astype(ml_dtypes.bfloat16)
    bcol = np.asarray(b, np.float32).reshape(K, 1)
    cent = np.asarray(centroids, np.float32)
    ident = np.eye(K, dtype=np.float32)
    return [
        dict(x=xaug[i * NB:(i + 1) * NB], xT=xT[i * NB:(i + 1) * NB],
             WT=WT, b=bcol, cent=cent, ident=ident)
        for i in range(NCORES)
    ]


def kernel(x, W, b, centroids):
    nc = _get_program()
    in_maps = make_in_maps(x, W, b, centroids)
    res = run_bass_kernel_spmd(nc, in_maps, list(range(NCORES)))
    return np.concatenate([res.results[i]["out"] for i in range(NCORES)],
                          axis=0).reshape(N, K * C)
